# revision 20
# baseline (speedup 1.0000x reference)
"""DLRM forward (embedding_lookup) Trainium2 Bass kernel.

Strategy: pure data-parallel over the batch (4096/8 = 512 samples per core);
every core holds the full (bf16) embedding table stack and all MLP weights.
Per 128-sample tile:
  - one indirect-DMA gather (f32 container of bf16 pairs) + 3 DVE adds to
    pool the L=4 bag slots -> es [128s, 26*64] bf16.
  - bottom MLP feature-major; layer-0 bias folded into an appended ones row
    of xT; single merged ReLU per layer where the bias allows.
  - 13 packed PE transposes ([128,128] = two tables) -> entity-major
    Tf [64m, e*128 + s]; 8 strided PSUM->SBUF copies rebuild Tf.
  - per-sample Gram matmuls Z_s = Tcat_s @ Tcat_s^T on PE into packed PSUM.
  - tril extraction via 8 multi-level-AP copies/tile into an interleaved-box
    layout zt [32*((i-1)%4) + j, ((i-1)//4)*128 + s]; junk rows are zeroed
    by zero weights; row 31 holds ones so top-W0's bias rides the Gram
    weight matrix.
  - top MLP feature-major; final Sigmoid on ACT; store [1,128] per tile.
No collectives needed.
"""

import numpy as np
import ml_dtypes

B, T, L, NR, M = 4096, 26, 4, 100000, 64
E27 = T + 1                      # 27 entities (bottom output + 26 tables)
NCORES = 8
BC = B // NCORES                 # 512 samples per core
TILE = 128
NT = BC // TILE                  # 4 tiles per core
NIG = 7                          # ceil(26/4) i-groups in the zt layout

_BF = ml_dtypes.bfloat16

_prog_cache = {}


def build_program():
    import concourse.bass as bass
    import concourse.mybir as mybir
    import concourse.tile as tile
    from concourse import bacc
    from concourse.masks import make_identity
    from contextlib import ExitStack

    bf16 = mybir.dt.bfloat16
    f32 = mybir.dt.float32
    i32 = mybir.dt.int32
    Relu = mybir.ActivationFunctionType.Relu
    Sigmoid = mybir.ActivationFunctionType.Sigmoid

    nc = bacc.Bacc(
        "TRN2", target_bir_lowering=False, debug=False,
        num_devices=NCORES,
    )

    def din(name, shape, dt):
        return nc.dram_tensor(name, shape, dt, kind="ExternalInput").ap()

    # table as f32 container (bf16 pairs): the vector-indirect DMA path
    # quantizes index values through the transfer dtype — bf16 corrupts any
    # index > 256, f32 is exact below 2^24.
    table = din("table", [T * NR, M // 2], f32)
    xT = din("xT", [14, BC], bf16)          # row 13 = ones (bias trick)
    idx = din("idx", [BC, T * L], i32)
    wb0 = din("wb0", [14, 512], bf16)       # [bot W0^T; bot_b0]
    wb1 = din("wb1", [128, 1024], bf16)     # bot W1^T k-chunk packed
    wb2 = din("wb2", [128, 128], bf16)      # bot W2^T k-chunk packed
    wt0x = din("wt0x", [64, 512], bf16)     # top W0[:, :64]^T
    wt0z = din("wt0z", [128, NIG * 512], bf16)  # top W0[:, 64:]^T interleaved-box
    wt1 = din("wt1", [128, 1024], bf16)     # top W1^T k-chunk packed
    wt2 = din("wt2", [128, 2], bf16)        # top W2^T k-chunk packed
    bb1 = din("bb1", [128, 2], f32)
    bb2 = din("bb2", [64, 1], f32)
    bt1 = din("bt1", [128, 2], f32)
    bt2 = din("bt2", [1, 1], f32)
    out = nc.dram_tensor("out", [NT, TILE], f32, kind="ExternalOutput").ap()

    with tile.TileContext(nc) as tc, ExitStack() as ctx:
        wpool = ctx.enter_context(tc.tile_pool(name="weights", bufs=1))
        ipool = ctx.enter_context(tc.tile_pool(name="idx", bufs=3))
        xpool = ctx.enter_context(tc.tile_pool(name="xin", bufs=3))
        hpool = ctx.enter_context(tc.tile_pool(name="acts", bufs=4))
        tfpool = ctx.enter_context(tc.tile_pool(name="tf", bufs=2))
        zpool = ctx.enter_context(tc.tile_pool(name="ztril", bufs=1))
        opool = ctx.enter_context(tc.tile_pool(name="outs", bufs=4))
        # role-split PSUM pools: cross-tile reuse then only waits on the
        # same role's (early) consumer instead of the end of the prior tile.
        p_h0 = ctx.enter_context(tc.tile_pool(name="ps_h0", bufs=1, space="PSUM"))
        p_top = ctx.enter_context(tc.tile_pool(name="ps_top", bufs=1, space="PSUM"))
        tppool = ctx.enter_context(tc.tile_pool(name="tp_psum", bufs=2, space="PSUM"))
        gpool = ctx.enter_context(tc.tile_pool(name="gram_psum", bufs=2, space="PSUM"))

        # --- constants / weights (tiles now, DMAs issued after the first
        # gathers are in flight so they don't head-block the SP queue) ---
        t_wb0 = wpool.tile([14, 512], bf16)
        t_wb1 = wpool.tile([128, 1024], bf16)
        t_wb2 = wpool.tile([128, 128], bf16)
        t_wt0x = wpool.tile([64, 512], bf16)
        t_wt0z = wpool.tile([128, NIG * 512], bf16)
        t_wt1 = wpool.tile([128, 1024], bf16)
        t_wt2 = wpool.tile([128, 2], bf16)
        t_bb1 = wpool.tile([128, 2], f32)
        t_bb2 = wpool.tile([64, 1], f32)
        t_bt1 = wpool.tile([128, 2], f32)
        t_bt2 = wpool.tile([1, 1], f32)
        ident = wpool.tile([128, 128], bf16)
        make_identity(nc, ident[:])

        # persistent zt tiles (interleaved-box layout), two parity sets for
        # double buffering; zeroed once so unwritten slots stay zero, row 31
        # set to ones so W0z row 31 carries top_b0.
        zsets = []
        for par in range(2):
            zt_ = zpool.tile([128, NIG * TILE], bf16, name=f"zt{par}",
                             tag=f"zt{par}")
            nc.vector.memset(zt_[:], 0.0)
            # compute-engine APs need 32-aligned partition bases: set rows
            # 0:32 to ones instead of just row 31. Rows 0:27 are rewritten
            # by every tile's extraction before use, rows 27:30 multiply
            # zero weights; only row 31 (the bias ones-row) must be 1.0.
            nc.vector.memset(zt_[0:32, :], 1.0)
            zsets.append(zt_)

        stA = {}
        stB = {}

        def stage_A(t):
            """DMAs + gather: SP and Pool engines only."""
            rows = slice(t * TILE, (t + 1) * TILE)
            it = ipool.tile([TILE, T * L], i32)
            nc.sync.dma_start(it[:], idx[rows, :])
            es4 = ipool.tile([TILE, T * L * (M // 2)], f32, tag="es4")
            nc.gpsimd.indirect_dma_start(
                out=es4[:],
                out_offset=None,
                in_=table[:],
                in_offset=bass.IndirectOffsetOnAxis(ap=it[:], axis=0),
            )
            xt = xpool.tile([14, TILE], bf16)
            nc.sync.dma_start(xt[:], xT[:, rows])
            stA[t] = (es4, xt)

        def stage_B(t):
            """Pool+transpose the gather on PE, rebuild Tf, bottom MLP."""
            es4, xt = stA.pop(t)
            es4_b = es4[:].bitcast(bf16)   # [128, (l, t, m)] l-major layout
            tf = tfpool.tile([64, E27 * TILE], bf16)

            # per table-pair, 4 accumulating matmuls (lhsT = [128s, 2t x 64m]
            # slice of the raw gather at bag slot l, rhs = identity)
            # transpose AND sum the L bag slots into f32 PSUM. Groups of 3
            # pairs share one [128, 384] PSUM tile; two strided copies per
            # group rebuild entity-major Tf.
            pair0 = 0
            ncopy = 0
            for grp, npair in enumerate((3, 3, 3, 3, 1)):
                pt = tppool.tile([128, 384], f32, tag="pt")
                for k in range(npair):
                    pr = pair0 + k
                    for l in range(L):
                        c0 = (l * T + 2 * pr) * M
                        nc.tensor.matmul(
                            pt[:, k * 128:(k + 1) * 128],
                            lhsT=es4_b[:, c0:c0 + 2 * M],
                            rhs=ident[:],
                            start=(l == 0), stop=(l == L - 1))
                # top halves: tables 2*pr -> e = 1 + 2*pr
                srcp = pt[:].rearrange("p (pr s) -> p pr s", s=TILE)
                tf_pair = tf[:, TILE:].rearrange(
                    "p (pr es) -> p pr es", es=2 * TILE)
                for half, (p0, p1) in enumerate(((0, 64), (64, 128))):
                    dst = tf_pair[:, pair0:pair0 + npair,
                                  half * TILE:(half + 1) * TILE]
                    eng = (nc.scalar.copy if ncopy % 5 < 2
                           else nc.vector.tensor_copy)
                    eng(dst, srcp[p0:p1, 0:npair, :])
                    ncopy += 1
                pair0 += npair

            # bottom MLP (feature-major); h1 reuses h0's PSUM bank, bot2's
            # PSUM rides the spare bytes of the first gram-quarter tile
            # (gram depends on tf entity 0 anyway).
            h0 = hpool.tile([128, 512], bf16)
            ps0 = p_h0.tile([128, 512], f32, tag="ps")
            for ob in range(4):
                nc.tensor.matmul(ps0[:, ob * 128:(ob + 1) * 128],
                                 lhsT=t_wb0[:, ob * 128:(ob + 1) * 128],
                                 rhs=xt[:], start=True, stop=True)
            nc.scalar.activation(h0[:], ps0[:], Relu)
            h1 = hpool.tile([128, 256], bf16)
            for ob in range(2):
                for kc in range(4):
                    nc.tensor.matmul(
                        ps0[:, ob * 128:(ob + 1) * 128],
                        lhsT=t_wb1[:, kc * 256 + ob * 128: kc * 256 + (ob + 1) * 128],
                        rhs=h0[:, kc * 128:(kc + 1) * 128],
                        start=(kc == 0), stop=(kc == 3))
                nc.scalar.activation(h1[:, ob * 128:(ob + 1) * 128],
                                     ps0[:, ob * 128:(ob + 1) * 128],
                                     Relu, bias=t_bb1[:, ob:ob + 1])
            g_q0 = gpool.tile([128, 32 * 28 + TILE], f32, tag="gram")
            psb = g_q0[0:64, 32 * 28:32 * 28 + TILE]
            for kc in range(2):
                nc.tensor.matmul(psb,
                                 lhsT=t_wb2[:, kc * 64:(kc + 1) * 64],
                                 rhs=h1[:, kc * 128:(kc + 1) * 128],
                                 start=(kc == 0), stop=(kc == 1))
            nc.scalar.activation(tf[:, 0:TILE], psb,
                                 Relu, bias=t_bb2[:, 0:1])
            stB[t] = (tf, g_q0)

        def stage_C(t):
            """Per-sample Gram + tril extraction + top MLP."""
            tf, g_q0 = stB.pop(t)
            tf3 = tf[:].rearrange("p (e s) -> p s e", s=TILE)
            zt = zsets[t % 2]
            ztv = zt[:].rearrange("p (ig qq sl) -> p sl ig qq",
                                  ig=NIG, qq=4, sl=32)
            for q in range(4):
                g = g_q0 if q == 0 else gpool.tile(
                    [128, 32 * 28 + TILE], f32, tag="gram")
                for sl in range(32):
                    s = q * 32 + sl
                    col = 28 * sl
                    sap = tf3[:, s:s + 1, :]
                    nc.tensor.matmul(g[0:E27, col:col + E27], lhsT=sap,
                                     rhs=sap, start=True, stop=True)
                # extraction: pair (i, j<i) -> zt[32*((i-1)%4) + j,
                # ((i-1)//4)*128 + q*32 + s'];  i = 1 + 4*ig + b
                g4 = g[0:E27, 0:32 * 28].rearrange(
                    "p (sl rb r4) -> p sl rb r4", sl=32, rb=7, r4=4)
                for b, (srb, dig) in enumerate(
                        (((0, 7), (0, 7)), ((0, 7), (0, 7)),
                         ((0, 6), (0, 6)), ((1, 7), (0, 6)))):
                    r4 = (1 + b) % 4
                    src = g4[:, :, srb[0]:srb[1], r4:r4 + 1]
                    zb = ztv[32 * b:32 * b + E27]
                    dst = zb[:, :, dig[0]:dig[1], q:q + 1]
                    on_act = b == 3 or (b == 1 and q % 2 == 0)
                    eng = nc.scalar.copy if on_act else nc.vector.tensor_copy
                    eng(dst, src)

            # top MLP (feature-major)
            t0 = hpool.tile([128, 512], bf16)
            ps = p_top.tile([128, 512], f32, tag="ps")
            for ob in range(4):
                nc.tensor.matmul(ps[:, ob * 128:(ob + 1) * 128],
                                 lhsT=t_wt0x[:, ob * 128:(ob + 1) * 128],
                                 rhs=tf[:, 0:TILE], start=True, stop=False)
                for ig in range(NIG):
                    nc.tensor.matmul(
                        ps[:, ob * 128:(ob + 1) * 128],
                        lhsT=t_wt0z[:, (ig * 4 + ob) * 128:(ig * 4 + ob + 1) * 128],
                        rhs=zt[:, ig * TILE:(ig + 1) * TILE],
                        start=False, stop=(ig == NIG - 1))
            nc.scalar.activation(t0[:], ps[:], Relu)
            t1 = hpool.tile([128, 256], bf16)
            pst = p_top.tile([128, 512], f32, tag="ps")
            for ob in range(2):
                for kc in range(4):
                    nc.tensor.matmul(
                        pst[:, ob * 128:(ob + 1) * 128],
                        lhsT=t_wt1[:, kc * 256 + ob * 128: kc * 256 + (ob + 1) * 128],
                        rhs=t0[:, kc * 128:(kc + 1) * 128],
                        start=(kc == 0), stop=(kc == 3))
                nc.scalar.activation(t1[:, ob * 128:(ob + 1) * 128],
                                     pst[:, ob * 128:(ob + 1) * 128],
                                     Relu, bias=t_bt1[:, ob:ob + 1])
            pso = pst[:, 384:512]
            for kc in range(2):
                nc.tensor.matmul(pso[0:1, 0:TILE], lhsT=t_wt2[:, kc:kc + 1],
                                 rhs=t1[:, kc * 128:(kc + 1) * 128],
                                 start=(kc == 0), stop=(kc == 1))
            osb = opool.tile([1, TILE], f32)
            nc.scalar.activation(osb[:], pso[0:1, 0:TILE], Sigmoid,
                                 bias=t_bt2[:, 0:1])
            nc.sync.dma_start(out[t:t + 1, :], osb[:])

        # software-pipelined emission: engine queues execute in program
        # order, so interleave stages of consecutive tiles explicitly. The
        # first two tiles' input DMAs go ahead of the weight loads so the
        # first gather isn't head-blocked in the SP queue.
        stage_A(0)
        if NT > 1:
            stage_A(1)
        for t_, d_ in [(t_wb0, wb0), (t_wb1, wb1), (t_wb2, wb2), (t_wt0x, wt0x),
                       (t_wt0z, wt0z), (t_wt1, wt1), (t_wt2, wt2),
                       (t_bb1, bb1), (t_bb2, bb2), (t_bt1, bt1), (t_bt2, bt2)]:
            nc.sync.dma_start(t_[:], d_[:])
        for u in range(NT + 2):
            if 2 <= u < NT:
                stage_A(u)
            if u >= 2:
                stage_C(u - 2)
            if 1 <= u <= NT:
                stage_B(u - 1)

    nc.compile()
    return nc


def _pack_k(w):
    """[K, N] with K a multiple of 128 -> [128, (K//128)*N], chunk k at
    columns [k*N, (k+1)*N)."""
    K, N = w.shape
    return np.ascontiguousarray(
        w.reshape(K // 128, 128, N).transpose(1, 0, 2).reshape(128, -1))


def _host_inputs(dense_x, sparse_idx, emb_tables,
                 bot_W0, bot_b0, bot_W1, bot_b1, bot_W2, bot_b2,
                 top_W0, top_b0, top_W1, top_b1, top_W2, top_b2):
    f32 = np.float32
    table_bf = np.ascontiguousarray(emb_tables.reshape(T * NR, M)).astype(_BF)
    table = table_bf.view(f32)                                       # [T*NR, 32]
    flat_idx = (np.asarray(sparse_idx, dtype=np.int64)
                + (np.arange(T, dtype=np.int64) * NR)[None, :, None]).astype(np.int32)
    # l-major layout so a 2-table-same-slot gather slice is contiguous
    idx_tl = np.ascontiguousarray(
        flat_idx.transpose(0, 2, 1)).reshape(B, T * L)               # [B, 104]
    xTh = np.empty((14, B), f32)                                     # [14, B]
    xTh[:13] = np.asarray(dense_x, f32).T
    xTh[13] = 1.0

    # W0z into the interleaved-box layout: pair (i, j<i) at row
    # 32*((i-1)%4) + j, chunk col block (ig*4 + ob)*128 with ig=(i-1)//4.
    # Row 31 of the ig=0 blocks carries top_b0 (zt row 31 is ones).
    wt0z_full = np.asarray(top_W0, f32)[:, 64:]                      # [512, 351]
    wt0zI = np.zeros((128, NIG * 512), f32)
    p = 0
    for i in range(1, E27):
        ig, b = (i - 1) // 4, (i - 1) % 4
        for j in range(i):
            wt0zI[32 * b + j, ig * 512:(ig + 1) * 512] = wt0z_full[:, p]
            p += 1
    wt0zI[31, 0:512] = np.asarray(top_b0, f32)
    # reorder each 512-block into (ob, 128) chunk order expected by matmuls
    wt0zI = wt0zI.reshape(128, NIG, 4, 128).reshape(128, -1)

    wb0h = np.empty((14, 512), f32)
    wb0h[:13] = np.asarray(bot_W0, f32).T
    wb0h[13] = np.asarray(bot_b0, f32)

    shared = {
        "table": table,
        "wb0": wb0h.astype(_BF),
        "wb1": _pack_k(np.asarray(bot_W1, f32).T).astype(_BF),
        "wb2": _pack_k(np.asarray(bot_W2, f32).T).astype(_BF),
        "wt0x": np.ascontiguousarray(np.asarray(top_W0, f32)[:, :64].T).astype(_BF),
        "wt0z": np.ascontiguousarray(wt0zI).astype(_BF),
        "wt1": _pack_k(np.asarray(top_W1, f32).T).astype(_BF),
        "wt2": _pack_k(np.asarray(top_W2, f32).T).astype(_BF),
        "bb1": np.ascontiguousarray(np.asarray(bot_b1, f32).reshape(2, 128).T),
        "bb2": np.asarray(bot_b2, f32).reshape(64, 1).copy(),
        "bt1": np.ascontiguousarray(np.asarray(top_b1, f32).reshape(2, 128).T),
        "bt2": np.asarray(top_b2, f32).reshape(1, 1).copy(),
    }
    in_maps = []
    for c in range(NCORES):
        sl = slice(c * BC, (c + 1) * BC)
        m = dict(shared)
        m["xT"] = np.ascontiguousarray(xTh[:, sl]).astype(_BF)
        m["idx"] = np.ascontiguousarray(idx_tl[sl, :])
        in_maps.append(m)
    return in_maps


def kernel(**inputs):
    from concourse import bass_utils

    if "prog" not in _prog_cache:
        _prog_cache["prog"] = build_program()
    nc = _prog_cache["prog"]
    in_maps = _host_inputs(**inputs)
    res = bass_utils.run_bass_kernel_spmd(nc, in_maps, core_ids=list(range(NCORES)))
    outs = [r["out"].reshape(BC, 1) for r in res.results]
    return np.concatenate(outs, axis=0).astype(np.float32)


if __name__ == "__main__":
    prog = build_program()
    print("program built OK")
